# revision 6
# baseline (speedup 1.0000x reference)
"""GAT encoder Bass kernel for TRN2 — v2: PE-matmul attention logits.

Design:
- Attention logits on the TensorEngine via block-diagonal matmuls over
  plane-packed edge streams: rows interleaved across G=8 groups (row p ->
  group g=p%8, lane r=p//8; R=16 lanes), packed [r*7+j = 112 partitions,
  slots] (ea) and [r*6+j = 96, slots] (x_src + x_dst).
  lhsT[(r,j),(h*16+r)] = V[j,h] built on device from the weights (h-major
  output so the PSUM->row regroup is one clean 3D DMA per head).
- x_dst planes replace the per-tile a_dst broadcast-add and the validity
  plane: pad slots have all-zero planes -> logit 0 -> exp 1, corrected by
  subtracting a host-counted npad from the softmax denominators.
- PSUM logits -> bf16 staging (DVE/ACT copies) -> [128 rows, h*SBMAX+s]
  via 4 SBUF->SBUF DMAs per stream per block (HWDGE is 625ns/DMA, so DMA
  instruction count is minimized everywhere; weights ship as 2 blobs).
- MLP: bf16 matmuls, ELU via dual PSUM-accumulated matmuls (exp-part +
  relu-part) with the -1 folded into b1, no output transposes (output stays
  [32, T*128]; host unscrambles), interleaved with edge blocks.
"""

import numpy as np
import concourse.bass as bass
import concourse.mybir as mybir
import concourse.tile as tile
from concourse.bass import AP

F32 = mybir.dt.float32
BF16 = mybir.dt.bfloat16
AF = mybir.ActivationFunctionType
OP = mybir.AluOpType

P = 128
NEG_SLOPE = 0.2
G = 8          # groups (row p -> group p % G, lane p // G)
R = 16         # lanes per group
NJ_E = 7
NJ_X = 6       # 3 x_src + 3 x_dst
H = 4
LAT = 32
SBMAX = 512
PE_E = R * NJ_E   # 112
PE_X = R * NJ_X   # 96
RH = R * H        # 64


def patch_tile_epilogue():
    from concourse.tile import ScopedClock
    import bass_rust

    if getattr(tile.TileContext, "_gatk_patched", False):
        return

    orig_lower = tile.TileContext._lower_ordered_insts

    def _lower_ordered_insts(self, ordered):
        for bb_name, insts in list(ordered.items()):
            out = []
            for inst in insts:
                si = inst.sync_info
                if si is not None and si.on_wait and len(si.on_wait) > 1:
                    waits = list(si.on_wait)
                    for i, w in enumerate(waits[:-1]):
                        n = bass_rust.InstNoOp(
                            name=f"{inst.name}-sw{i}", ins=[], outs=[])
                        n.engine = inst.engine
                        n.sync_info = mybir.SyncInfo(
                            on_wait=[w], on_update=[])
                        out.append(n)
                    si.on_wait.clear()
                    si.on_wait.append(waits[-1])
                out.append(inst)
            ordered[bb_name] = out
        return orig_lower(self, ordered)

    tile.TileContext._lower_ordered_insts = _lower_ordered_insts
    tile.TileContext._gatk_patched = True

    def _drain_and_barrier(self, tick_clock, wait_clock):
        drain_inst = self.nc.sync.drain()
        wait_clock.add_sem_waits(
            drain_inst.ins, ScopedClock({None: tick_clock.global_clock})
        )
        si = drain_inst.ins.sync_info
        waits = list(si.on_wait or [])
        si.on_wait.clear()
        for w in waits:
            n = self.nc.sync.nop()
            nsi = n.ins.sync_info
            if nsi is None:
                n.ins.sync_info = mybir.SyncInfo(on_wait=[w], on_update=[])
            else:
                nsi.on_wait.append(w)
        self.nc.all_engine_barrier()
        assert self.sems is not None
        popped = self.nc._tile_sem_poison_stack.pop()
        assert popped is self._sem_poison
        self.nc.clear_and_free_semaphores(list(self.sems.allocated().values()))
        self.nc.all_engine_barrier()

    tile.TileContext._drain_and_barrier = _drain_and_barrier


# ---------------------------------------------------------------------------
# Host-side sharding / layout prep (pure indexing + input redistribution).
# ---------------------------------------------------------------------------
def host_prep(x, edge_index, edge_attr, n_cores):
    import ml_dtypes
    bf16 = ml_dtypes.bfloat16

    N = x.shape[0]
    E = edge_index.shape[1]
    NLOC = N // n_cores
    NPAD = ((NLOC + P - 1) // P) * P
    T = NPAD // P

    src = np.asarray(edge_index[0], dtype=np.int64)
    dst = np.asarray(edge_index[1], dtype=np.int64)
    x = np.asarray(x, dtype=np.float32)
    ea = np.asarray(edge_attr, dtype=np.float32)

    deg = np.bincount(dst, minlength=N).astype(np.int64)

    orders = np.zeros((n_cores, NPAD), dtype=np.int64)
    ranks = np.zeros((n_cores, NPAD), dtype=np.int64)
    degp = np.zeros((n_cores, NPAD), dtype=np.int64)
    for c in range(n_cores):
        dloc = np.zeros(NPAD, dtype=np.int64)
        dloc[:NLOC] = deg[c * NLOC:(c + 1) * NLOC]
        dloc[NLOC:] = -1
        o = np.argsort(dloc, kind="stable")
        orders[c] = o
        ranks[c, o] = np.arange(NPAD)
        degp[c] = np.maximum(dloc[o], 0)

    D = np.zeros(T, dtype=np.int64)
    for t in range(T):
        D[t] = degp[:, t * P:(t + 1) * P].max() + 1
    off = np.zeros(T + 1, dtype=np.int64)
    off[1:] = np.cumsum(D)
    S = int(off[-1])

    e_core = dst // NLOC
    e_rank = ranks[e_core, dst - e_core * NLOC]
    e_p = e_rank % P
    e_t = e_rank // P
    order_e = np.argsort(dst, kind="stable")
    kk = np.empty(E, dtype=np.int64)
    ds = dst[order_e]
    grp_start = np.r_[0, np.flatnonzero(ds[1:] != ds[:-1]) + 1]
    lengths = np.diff(np.r_[grp_start, E])
    within = np.arange(E) - np.repeat(grp_start, lengths)
    kk[order_e] = within + 1
    e_s = off[e_t] + kk

    e_g = e_p % G
    e_r = e_p // G

    ea_pk = np.zeros((n_cores, PE_E, G * S), dtype=np.float32)
    x_pk = np.zeros((n_cores, PE_X, G * S), dtype=np.float32)
    xgb_row = np.zeros((n_cores, P, 3 * S), dtype=np.float32)

    for j in range(NJ_E):
        ea_pk[e_core, e_r * NJ_E + j, e_g * S + e_s] = ea[:, j]
    for j in range(3):
        x_pk[e_core, e_r * NJ_X + j, e_g * S + e_s] = x[src, j]
        x_pk[e_core, e_r * NJ_X + 3 + j, e_g * S + e_s] = x[dst, j]
        xgb_row[e_core, e_p, j * S + e_s] = x[src, j]

    invd = np.zeros((n_cores, P, T), dtype=np.float32)
    npad = np.zeros((n_cores, P, T), dtype=np.float32)
    node_of = np.zeros((n_cores, T, P), dtype=np.int64)
    pp = np.arange(P)
    gg = pp % G
    rr = pp // G
    for c in range(n_cores):
        loc = orders[c]
        glob = c * NLOC + loc
        valid = loc < NLOC
        xg_nodes = np.where(valid[:, None], x[np.minimum(glob, N - 1)], 0.0)
        for t in range(T):
            sl = slice(t * P, (t + 1) * P)
            xn = xg_nodes[sl]                       # [P, 3]
            s0 = off[t]
            for j in range(3):
                x_pk[c, rr * NJ_X + j, gg * S + s0] = xn[:, j]
                x_pk[c, rr * NJ_X + 3 + j, gg * S + s0] = xn[:, j]
                xgb_row[c, :, j * S + s0] = xn[:, j]
            invd[c, :, t] = 1.0 / np.maximum(degp[c, sl], 1)
            npad[c, :, t] = D[t] - 1 - degp[c, sl]
            node_of[c, t] = glob[sl]

    sched = dict(T=T, D=D, off=off, S=S, NLOC=NLOC, NPAD=NPAD,
                 n_cores=n_cores)
    streams = dict(ea_pk=ea_pk.astype(bf16), x_pk=x_pk.astype(bf16),
                   xgb_row=xgb_row.astype(bf16), invd=invd, npad=npad)
    unscr = dict(node_of=node_of, valid_loc=orders < NLOC)
    return sched, streams, unscr


def host_weights(sched, W_gat, att_src, att_dst, W_edge, att_edge,
                 bias_gat, W1, b1, prelu_a, W2, b2):
    """Pure-layout reshapes/replications/casts packed into two blobs."""
    import ml_dtypes
    bf16 = ml_dtypes.bfloat16
    HC = P
    nH, C = att_src.shape

    hmask = np.zeros((HC, nH), dtype=np.float32)
    for h in range(nH):
        hmask[h * C:(h + 1) * C, h] = 1.0

    q_ea = np.zeros((P, PE_E), dtype=np.float32)
    rm_ea = np.zeros((P, RH), dtype=np.float32)
    q_xs = np.zeros((P, PE_X), dtype=np.float32)
    q_xd = np.zeros((P, PE_X), dtype=np.float32)
    rm_x = np.zeros((P, RH), dtype=np.float32)
    for r in range(R):
        for j in range(NJ_E):
            q_ea[j, r * NJ_E + j] = 1.0
            for h in range(nH):
                rm_ea[r * NJ_E + j, h * R + r] = 1.0
        for j in range(3):
            q_xs[j, r * NJ_X + j] = 1.0
            q_xd[j, r * NJ_X + 3 + j] = 1.0
        for j in range(NJ_X):
            for h in range(nH):
                rm_x[r * NJ_X + j, h * R + r] = 1.0

    nj = W_gat.shape[0]
    wpj = np.zeros((nj * nH, HC), dtype=np.float32)
    for h in range(nH):
        wpj[nj * h: nj * (h + 1), C * h: C * (h + 1)] = \
            W_gat[:, C * h: C * (h + 1)]

    def colpad(a):
        out = np.zeros((P, a.shape[1]), dtype=np.float32)
        out[:a.shape[0]] = a
        return out

    fparts = dict(
        wgT=colpad(np.ascontiguousarray(W_gat.T, np.float32)),
        weT=colpad(np.ascontiguousarray(W_edge.T, np.float32)),
        asc=colpad(att_src.reshape(HC, 1).astype(np.float32)),
        adc=colpad(att_dst.reshape(HC, 1).astype(np.float32)),
        aec=colpad(att_edge.reshape(HC, 1).astype(np.float32)),
        hmask=hmask, q_ea=q_ea, rm_ea=rm_ea, q_xs=q_xs, q_xd=q_xd,
        rm_x=rm_x,
        bgc=colpad(bias_gat.reshape(HC, 1).astype(np.float32)),
        b1c=colpad(b1.reshape(HC, 1).astype(np.float32)),
        b2c=colpad(b2.reshape(LAT, 1).astype(np.float32)),
    )
    foff = {}
    c0 = 0
    for k, a in fparts.items():
        foff[k] = c0
        c0 += a.shape[1]
    blob_f = np.concatenate(list(fparts.values()), axis=1)

    bparts = dict(
        wpj=colpad(wpj),
        w1=np.ascontiguousarray(W1, np.float32),
        w2=colpad(np.ascontiguousarray(W2, np.float32)),
        ones=np.ones((P, 1), np.float32),
        ident=np.eye(P, dtype=np.float32),
    )
    boff = {}
    c0 = 0
    for k, a in bparts.items():
        boff[k] = c0
        c0 += a.shape[1]
    blob_b = np.concatenate(list(bparts.values()), axis=1).astype(bf16)

    w = dict(blob_f=blob_f, blob_b=blob_b)
    return w, foff, boff, blob_f.shape[1], blob_b.shape[1]


# ---------------------------------------------------------------------------
# Device program.
# ---------------------------------------------------------------------------
def build_program(sched, foff, boff, KF, KB, prelu_alpha=0.25, chunk_tiles=4):
    T = sched["T"]
    D = sched["D"]
    off = sched["off"]
    S = sched["S"]
    HC = P

    blocks = []
    t0 = 0
    caps = [SBMAX] * 100
    bi = 0
    while t0 < T:
        t1 = t0
        acc = 0
        while t1 < T and acc + int(D[t1]) <= caps[bi]:
            acc += int(D[t1])
            t1 += 1
        assert t1 > t0
        blocks.append((t0, t1))
        t0 = t1
        bi += 1

    def runs_of(t0, t1):
        runs = []
        a = t0
        while a < t1:
            b = a
            while b < t1 and D[b] == D[a]:
                b += 1
            runs.append((a, b))
            a = b
        return runs

    Dmax = int(D.max())

    nc = bass.Bass()
    dt = F32

    ea_d = nc.dram_tensor("ea_pk", [PE_E, G * S], BF16, kind="ExternalInput")
    x_d = nc.dram_tensor("x_pk", [PE_X, G * S], BF16, kind="ExternalInput")
    xgb_d = nc.dram_tensor("xgb_row", [P, 3 * S], BF16, kind="ExternalInput")
    invd_d = nc.dram_tensor("invd", [P, T], dt, kind="ExternalInput")
    npad_d = nc.dram_tensor("npad", [P, T], dt, kind="ExternalInput")
    bf_d = nc.dram_tensor("blob_f", [P, KF], dt, kind="ExternalInput")
    bb_d = nc.dram_tensor("blob_b", [P, KB], BF16, kind="ExternalInput")
    out_d = nc.dram_tensor("out", [LAT, T * P], dt, kind="ExternalOutput")

    with tile.TileContext(nc) as tc:
        with (
            tc.tile_pool(name="wp", bufs=1) as wp,
            tc.tile_pool(name="sp", bufs=2) as sp,
            tc.tile_pool(name="mp", bufs=2) as mp,
            tc.tile_pool(name="pl", bufs=2, space="PSUM") as pl,
            tc.tile_pool(name="pt", bufs=2, space="PSUM") as ptp,
            tc.tile_pool(name="p1", bufs=1, space="PSUM") as p1p,
            tc.tile_pool(name="p2", bufs=1, space="PSUM") as p2p,
            tc.tile_pool(name="p3", bufs=1, space="PSUM") as p3p,
        ):
            # ---------------- phase 0: weights & derived ----------------
            blf = wp.tile([P, KF], dt, tag="blf")
            blb = wp.tile([P, KB], BF16, tag="blb")
            ivd = wp.tile([P, T], dt, tag="ivd")
            npd = wp.tile([P, T], dt, tag="npd")
            nc.sync.dma_start(blf[:], bf_d[:])
            nc.sync.dma_start(blb[:], bb_d[:])
            nc.sync.dma_start(ivd[:], invd_d[:])
            nc.sync.dma_start(npd[:], npad_d[:])

            def fsl(name, p1_, width):
                return blf[0:p1_, foff[name]:foff[name] + width]

            def bsl(name, p1_, width):
                return blb[0:p1_, boff[name]:boff[name] + width]

            wgT = fsl("wgT", P, 3)
            weT = fsl("weT", P, NJ_E)
            asc = fsl("asc", P, 1)
            adc = fsl("adc", P, 1)
            aec = fsl("aec", P, 1)
            hma = fsl("hmask", P, H)
            q_ea = fsl("q_ea", NJ_E, PE_E)
            rm_ea = fsl("rm_ea", PE_E, RH)
            q_xs = fsl("q_xs", 3, PE_X)
            q_xd = fsl("q_xd", 3, PE_X)
            rm_x = fsl("rm_x", PE_X, RH)
            bgc = fsl("bgc", P, 1)
            b1c = fsl("b1c", P, 1)
            b2c = fsl("b2c", LAT, 1)
            wpj = bsl("wpj", 3 * H, HC)
            w1s = bsl("w1", P, HC)
            w2s = bsl("w2", P, LAT)
            onec = bsl("ones", P, 1)
            ident = bsl("ident", P, P)

            # rhsm[:, 0:4]=aec*hm, [4:8]=asc*hm, [8:12]=adc*hm
            rhsm = wp.tile([HC, 3 * H], dt, tag="rhsm")
            for i, col in enumerate([aec, asc, adc]):
                cb = AP(col.tensor, col.offset,
                        [list(col.ap[0]), [0, H]])
                nc.vector.tensor_tensor(
                    out=rhsm[:, i * H:(i + 1) * H], in0=cb, in1=hma,
                    op=OP.mult)

            psV = p1p.tile([NJ_E, H], dt, tag="ps1")
            nc.tensor.matmul(psV[:], weT, rhsm[:, 0:H], start=True,
                             stop=True)
            v7h = wp.tile([NJ_E, H], dt, tag="v7h")
            nc.vector.tensor_copy(v7h[:], psV[:])
            psU = p1p.tile([3, 2 * H], dt, tag="ps1")
            nc.tensor.matmul(psU[:], wgT, rhsm[:, H:3 * H], start=True,
                             stop=True)
            u3x = wp.tile([3, 2 * H], dt, tag="u3x")
            nc.vector.tensor_copy(u3x[:], psU[:])

            # lhsT build: free index f = h*R + r  (h-major)
            ps_ea = pl.tile([PE_E, RH], dt, tag="pslog")
            v7b = AP(v7h[:].tensor, v7h[:].offset,
                     [list(v7h[:].ap[0]), [1, H], [0, R]])
            nc.tensor.matmul(ps_ea[:], q_ea, v7b, start=True, stop=True)
            lhsT_ea = wp.tile([PE_E, RH], BF16, tag="lhsT_ea")
            nc.vector.tensor_tensor(out=lhsT_ea[:], in0=ps_ea[:],
                                    in1=rm_ea, op=OP.mult)
            ps_x = pl.tile([PE_X, RH], dt, tag="pslog")
            usb = AP(u3x[:].tensor, u3x[:].offset,
                     [list(u3x[:].ap[0]), [1, H], [0, R]])
            udb = AP(u3x[:].tensor, u3x[:].offset + H,
                     [list(u3x[:].ap[0]), [1, H], [0, R]])
            nc.tensor.matmul(ps_x[:], q_xs, usb, start=True, stop=False)
            nc.tensor.matmul(ps_x[:], q_xd, udb, start=False, stop=True)
            lhsT_x = wp.tile([PE_X, RH], BF16, tag="lhsT_x")
            nc.vector.tensor_tensor(out=lhsT_x[:], in0=ps_x[:], in1=rm_x,
                                    op=OP.mult)

            # b1' = b1 - W1^T 1
            psb1 = p1p.tile([HC, 1], dt, tag="ps1")
            nc.tensor.matmul(psb1[:], w1s, onec, start=True, stop=True)
            b1p = wp.tile([HC, 1], dt, tag="b1p")
            nc.vector.tensor_tensor(out=b1p[:], in0=b1c, in1=psb1[:],
                                    op=OP.subtract)

            den_all = wp.tile([P, H * T], dt, tag="den_all")
            rec_all = wp.tile([P, H * T], dt, tag="rec_all")
            agg_all = wp.tile([P, 3 * H * T], dt, tag="agg_all")
            agg_nrm = wp.tile([P, 3 * H * T], BF16, tag="agg_nrm")
            out_sb = wp.tile([LAT, T * P], dt, tag="out_sb")

            # ---------------- main loop over blocks ----------------
            for (bt0, bt1) in blocks:
                o0, o1 = int(off[bt0]), int(off[bt1])
                SB = o1 - o0

                ea_t = sp.tile([PE_E, G * SBMAX], BF16, tag="ea_t")
                x_t = sp.tile([PE_X, G * SBMAX], BF16, tag="x_t")
                xgb = sp.tile([P, 3 * SBMAX], BF16, tag="xgb")
                sbA = sp.tile([RH, G * SBMAX], BF16, tag="sbA")
                sbB = sp.tile([RH, G * SBMAX], BF16, tag="sbB")
                eaL = sp.tile([P, H * SBMAX], BF16, tag="eaL")
                xL = sp.tile([P, H * SBMAX], BF16, tag="xL")
                exb = sp.tile([P, H * SBMAX], BF16, tag="exb")

                nc.sync.dma_start(
                    AP(ea_t[:].tensor, ea_t[:].offset,
                       [list(ea_t[:].ap[0]), [SBMAX, G], [1, SB]]),
                    AP(ea_d[:].tensor, ea_d[:].offset + o0,
                       [list(ea_d[:].ap[0]), [S, G], [1, SB]]))
                nc.sync.dma_start(
                    AP(x_t[:].tensor, x_t[:].offset,
                       [list(x_t[:].ap[0]), [SBMAX, G], [1, SB]]),
                    AP(x_d[:].tensor, x_d[:].offset + o0,
                       [list(x_d[:].ap[0]), [S, G], [1, SB]]))
                nc.sync.dma_start(
                    AP(xgb[:].tensor, xgb[:].offset,
                       [list(xgb[:].ap[0]), [SBMAX, 3], [1, SB]]),
                    AP(xgb_d[:].tensor, xgb_d[:].offset + o0,
                       [list(xgb_d[:].ap[0]), [S, 3], [1, SB]]))

                # per-group PE logits -> bf16 staging (ea stream fully first
                # so the self-loop reduce can start before x finishes);
                # regroup: dst[r*G+g, h*SBMAX+s] = stg[h*R+r, g*SBMAX+s]
                copy_eng = [nc.scalar, nc.scalar]
                FA = G * SBMAX
                FO = H * SBMAX
                for g in range(G):
                    psA = pl.tile([RH, SBMAX], dt, tag="pslog")
                    nc.tensor.matmul(
                        psA[:, 0:SB], lhsT_ea[:],
                        ea_t[0:PE_E, g * SBMAX:g * SBMAX + SB],
                        start=True, stop=True)
                    ce = copy_eng[g % 2]
                    dst = sbA[:, g * SBMAX:g * SBMAX + SB]
                    if ce is nc.scalar:
                        ce.copy(dst, psA[:, 0:SB])
                    else:
                        ce.tensor_copy(dst, psA[:, 0:SB])
                for h in range(H):
                    st = sbA[:]
                    dl = eaL[:]
                    nc.sync.dma_start(
                        AP(dl.tensor, dl.offset + h * SBMAX,
                           [[FO, R * G], [1, SB]]),
                        AP(st.tensor, st.offset + h * R * FA,
                           [[FA, R], [SBMAX, G], [1, SB]]))
                for g in range(G):
                    psB = pl.tile([RH, SBMAX], dt, tag="pslog")
                    nc.tensor.matmul(
                        psB[:, 0:SB], lhsT_x[:],
                        x_t[0:PE_X, g * SBMAX:g * SBMAX + SB],
                        start=True, stop=True)
                    ce = copy_eng[(g + 1) % 2]
                    dst = sbB[:, g * SBMAX:g * SBMAX + SB]
                    if ce is nc.scalar:
                        ce.copy(dst, psB[:, 0:SB])
                    else:
                        ce.tensor_copy(dst, psB[:, 0:SB])
                for h in range(H):
                    st = sbB[:]
                    dl = xL[:]
                    nc.sync.dma_start(
                        AP(dl.tensor, dl.offset + h * SBMAX,
                           [[FO, R * G], [1, SB]]),
                        AP(st.tensor, st.offset + h * R * FA,
                           [[FA, R], [SBMAX, G], [1, SB]]))

                eal = eaL[:]
                xll = xL[:]
                exl = exb[:]
                xgl = xgb[:]

                # self-loop ea mean (batched over same-D runs)
                for (ta, tb) in runs_of(bt0, bt1):
                    nt = tb - ta
                    dt_t = int(D[ta])
                    lt = int(off[ta]) - o0
                    aes = mp.tile([P, H * nt], dt, tag="aes")
                    nc.vector.tensor_reduce(
                        out=AP(aes[:].tensor, aes[:].offset,
                               [list(aes[:].ap[0]), [nt, H], [1, nt]]),
                        in_=AP(eal.tensor, eal.offset + lt,
                               [list(eal.ap[0]), [SBMAX, H], [dt_t, nt],
                                [1, dt_t]]),
                        axis=mybir.AxisListType.X, op=OP.add)
                    nc.vector.tensor_tensor(
                        out=AP(eal.tensor, eal.offset + lt,
                               [list(eal.ap[0]), [SBMAX, H], [dt_t, nt]]),
                        in0=AP(aes[:].tensor, aes[:].offset,
                               [list(aes[:].ap[0]), [nt, H], [1, nt]]),
                        in1=AP(ivd[:].tensor, ivd[:].offset + ta,
                               [list(ivd[:].ap[0]), [0, H], [1, nt]]),
                        op=OP.mult)

                # logits = eaL + xL (2x bf16), lrelu, exp
                nc.vector.tensor_tensor(
                    out=AP(xll.tensor, xll.offset,
                           [list(xll.ap[0]), [SBMAX, H], [1, SB]]),
                    in0=AP(xll.tensor, xll.offset,
                           [list(xll.ap[0]), [SBMAX, H], [1, SB]]),
                    in1=AP(eal.tensor, eal.offset,
                           [list(eal.ap[0]), [SBMAX, H], [1, SB]]),
                    op=OP.add)
                nc.scalar.activation(
                    AP(xll.tensor, xll.offset,
                       [list(xll.ap[0]), [SBMAX, H], [1, SB]]),
                    AP(xll.tensor, xll.offset,
                       [list(xll.ap[0]), [SBMAX, H], [1, SB]]),
                    AF.Prelu, alpha=NEG_SLOPE)
                nc.scalar.activation(
                    AP(exl.tensor, exl.offset,
                       [list(exl.ap[0]), [SBMAX, H], [1, SB]]),
                    AP(xll.tensor, xll.offset,
                       [list(xll.ap[0]), [SBMAX, H], [1, SB]]),
                    AF.Exp)

                # denominators (batched over same-D runs)
                for (ta, tb) in runs_of(bt0, bt1):
                    nt = tb - ta
                    dt_t = int(D[ta])
                    lt = int(off[ta]) - o0
                    nc.vector.tensor_reduce(
                        out=AP(den_all[:].tensor, den_all[:].offset + ta,
                               [list(den_all[:].ap[0]), [T, H], [1, nt]]),
                        in_=AP(exl.tensor, exl.offset + lt,
                               [list(exl.ap[0]), [SBMAX, H], [dt_t, nt],
                                [1, dt_t]]),
                        axis=mybir.AxisListType.X, op=OP.add)

                # weighted aggregation, batched over same-D runs
                for (ta, tb) in runs_of(bt0, bt1):
                    dt_t = int(D[ta])
                    ra = ta
                    while ra < tb:
                        # cap the run chunk so the msg scratch stays small
                        max_nt = max(1, (3 * Dmax) // dt_t)
                        rb = min(ra + max_nt, tb)
                        nt = rb - ra
                        dn = dt_t * nt
                        lt = int(off[ra]) - o0
                        msg = mp.tile([P, H * 3 * 3 * Dmax + 64], BF16,
                                      tag="msg")
                        m_ap = AP(msg[:].tensor, msg[:].offset,
                                  [list(msg[:].ap[0]), [3 * dn, H],
                                   [dn, 3], [1, dn]])
                        ealpha = AP(exl.tensor, exl.offset + lt,
                                    [list(exl.ap[0]), [SBMAX, H], [0, 3],
                                     [1, dn]])
                        xgs = AP(xgl.tensor, xgl.offset + lt,
                                 [list(xgl.ap[0]), [0, H], [SBMAX, 3],
                                  [1, dn]])
                        nc.vector.tensor_tensor(out=m_ap, in0=ealpha,
                                                in1=xgs, op=OP.mult)
                        nc.vector.tensor_reduce(
                            out=AP(agg_all[:].tensor,
                                   agg_all[:].offset + ra,
                                   [list(agg_all[:].ap[0]), [T, H * 3],
                                    [1, nt]]),
                            in_=AP(msg[:].tensor, msg[:].offset,
                                   [list(msg[:].ap[0]), [dn, H * 3],
                                    [dt_t, nt], [1, dt_t]]),
                            axis=mybir.AxisListType.X, op=OP.add)
                        ra = rb

                # ---- block tail: den corr + norm + MLP
                nt_b = bt1 - bt0
                nc.vector.tensor_tensor(
                    out=AP(den_all[:].tensor, den_all[:].offset + bt0,
                           [list(den_all[:].ap[0]), [T, H], [1, nt_b]]),
                    in0=AP(den_all[:].tensor, den_all[:].offset + bt0,
                           [list(den_all[:].ap[0]), [T, H], [1, nt_b]]),
                    in1=AP(npd[:].tensor, npd[:].offset + bt0,
                           [list(npd[:].ap[0]), [0, H], [1, nt_b]]),
                    op=OP.subtract)
                nc.vector.reciprocal(
                    AP(rec_all[:].tensor, rec_all[:].offset + bt0,
                       [list(rec_all[:].ap[0]), [T, H], [1, nt_b]]),
                    AP(den_all[:].tensor, den_all[:].offset + bt0,
                       [list(den_all[:].ap[0]), [T, H], [1, nt_b]]))
                # agg_nrm[t*12 + h*3 + j] = agg_all[(h*3+j)*T + t] * rec
                nc.vector.tensor_tensor(
                    out=AP(agg_nrm[:].tensor, agg_nrm[:].offset + bt0 * 12,
                           [list(agg_nrm[:].ap[0]), [3, H], [1, 3],
                            [12, nt_b]]),
                    in0=AP(agg_all[:].tensor, agg_all[:].offset + bt0,
                           [list(agg_all[:].ap[0]), [3 * T, H], [T, 3],
                            [1, nt_b]]),
                    in1=AP(rec_all[:].tensor, rec_all[:].offset + bt0,
                           [list(rec_all[:].ap[0]), [T, H], [0, 3],
                            [1, nt_b]]),
                    op=OP.mult)

                ca = bt0
                while ca < bt1:
                    cb = min(ca + chunk_tiles, bt1)
                    ncw = (cb - ca) * P
                    psT = ptp.tile([3 * H, chunk_tiles * P], BF16, tag="psT")
                    for k, ti in enumerate(range(ca, cb)):
                        nc.tensor.transpose(
                            out=psT[:, k * P:(k + 1) * P],
                            in_=agg_nrm[:, ti * 12:(ti + 1) * 12],
                            identity=ident)
                    aggT = mp.tile([3 * H, chunk_tiles * P], BF16, tag="aggT")
                    nc.scalar.copy(aggT[:, :ncw], psT[:, :ncw])

                    ps1 = p1p.tile([HC, chunk_tiles * P], dt, tag="ps1")
                    nc.tensor.matmul(ps1[:, :ncw], wpj, aggT[:, :ncw],
                                     start=True, stop=True)
                    ub = mp.tile([HC, chunk_tiles * P], BF16, tag="ub")
                    rb = mp.tile([HC, chunk_tiles * P], BF16, tag="rb")
                    nc.scalar.activation(ub[:, :ncw], ps1[:, :ncw], AF.Exp,
                                         bias=bgc)
                    nc.vector.tensor_scalar(
                        out=rb[:, :ncw], in0=ps1[:, :ncw], scalar1=bgc,
                        scalar2=0.0, op0=OP.add, op1=OP.max)
                    nc.vector.tensor_scalar(
                        out=ub[:, :ncw], in0=ub[:, :ncw], scalar1=1.0,
                        scalar2=None, op0=OP.min)

                    ps2 = p2p.tile([HC, chunk_tiles * P], dt, tag="ps2")
                    nc.tensor.matmul(ps2[:, :ncw], w1s, ub[:, :ncw],
                                     start=True, stop=False)
                    nc.tensor.matmul(ps2[:, :ncw], w1s, rb[:, :ncw],
                                     start=False, stop=True)
                    h2 = mp.tile([HC, chunk_tiles * P], BF16, tag="h2")
                    nc.scalar.activation(h2[:, :ncw], ps2[:, :ncw], AF.Prelu,
                                         bias=b1p[:, 0:1], alpha=prelu_alpha)

                    ps3 = p3p.tile([LAT, chunk_tiles * P], dt, tag="ps3")
                    nc.tensor.matmul(ps3[:, :ncw], w2s, h2[:, :ncw],
                                     start=True, stop=True)
                    nc.scalar.activation(
                        out_sb[:, ca * P:ca * P + ncw], ps3[:, :ncw],
                        AF.Identity, bias=b2c)
                    ca = cb

                nc.sync.dma_start(out_d[:, bt0 * P:bt1 * P],
                                  out_sb[:, bt0 * P:bt1 * P])

    return nc


# ---------------------------------------------------------------------------
def make_in_maps(sched, streams, w, n_cores):
    maps = []
    for c in range(n_cores):
        m = dict(
            ea_pk=streams["ea_pk"][c], x_pk=streams["x_pk"][c],
            xgb_row=streams["xgb_row"][c], invd=streams["invd"][c],
            npad=streams["npad"][c],
        )
        m.update(w)
        maps.append(m)
    return maps


def unscramble(results, sched, unscr, N):
    n_cores = sched["n_cores"]
    T = sched["T"]
    out = np.zeros((N, LAT), dtype=np.float32)
    for c in range(n_cores):
        o = results[c]["out"].reshape(LAT, T, P).transpose(1, 2, 0)
        node_of = unscr["node_of"][c]
        valid = unscr["valid_loc"][c].reshape(T, P)
        for t in range(T):
            v = valid[t]
            out[node_of[t][v]] = o[t][v]
    return out


# ---------------------------------------------------------------------------
_CACHE = {}


def kernel(x, edge_index, edge_attr, W_gat, att_src, att_dst, W_edge,
           att_edge, bias_gat, W1, b1, prelu_a, W2, b2):
    from concourse.bass_utils import run_bass_kernel_spmd

    patch_tile_epilogue()
    n_cores = 8
    x = np.asarray(x)
    edge_index = np.asarray(edge_index)
    edge_attr = np.asarray(edge_attr)

    sched, streams, unscr = host_prep(x, edge_index, edge_attr, n_cores)
    w, foff, boff, KF, KB = host_weights(
        sched, np.asarray(W_gat), np.asarray(att_src), np.asarray(att_dst),
        np.asarray(W_edge), np.asarray(att_edge), np.asarray(bias_gat),
        np.asarray(W1), np.asarray(b1), np.asarray(prelu_a), np.asarray(W2),
        np.asarray(b2))

    key = (sched["T"], sched["S"], tuple(int(d) for d in sched["D"]),
           float(np.asarray(prelu_a)))
    if key not in _CACHE:
        _CACHE[key] = build_program(sched, foff, boff, KF, KB,
                                    prelu_alpha=float(np.asarray(prelu_a)))
    nc = _CACHE[key]

    maps = make_in_maps(sched, streams, w, n_cores)
    res = run_bass_kernel_spmd(nc, maps, core_ids=list(range(n_cores)))
    out = unscramble(res.results, sched, unscr, x.shape[0])
    return out.astype(np.float32)
